# revision 4
# baseline (speedup 1.0000x reference)
"""Trainium2 Bass kernel for the quantized BasicBlock (nn_BasicBlock_15436112462307).

Strategy
--------
Data-parallel over batch: 64 images -> 8 cores x 8 images. Weights/BN replicated.

fake_quant makes every conv operand an exact small integer (-7..7) times a
global fp32 scale.  We factor the scales out on the host and feed pure
integers to the PE as fp8e4 (integers <=7 are exact in fp8e4), using
perf_mode=DoubleRow so one matmul contracts all 256 input channels
(lhsT [128,2,128] / rhs [128,2,N]) at 2x fp8 rate.  PSUM accumulates the
integer dot products exactly in fp32, so the conv itself is EXACT; all
rounding happens only in the per-channel epilogues, which replicate the
reference's fp32 arithmetic.

Spatial layout: each 28x28 image is zero-padded to 30x30 and flattened, so
every 3x3 conv tap is a pure diagonal shift in the flat index -> conv =
9 accumulating matmuls over contiguous windows.  We compute 30-wide output
rows (2 garbage columns per row) and discard the garbage in the epilogue APs.

Epilogue 1 (conv1 -> conv2 input):  q2 = rne(clip(P1*(7*sx*sw1*inv1) + 7*b1, +-7))
using the fp32 magic-number trick (+-1.5*2^23) for round-to-nearest-even;
the result is an exact integer written directly as fp8 into the padded conv2
input buffer.  The activation fake-quant scale alpha2 = max|hardtanh(...)| is
1.0 whenever anything clips (always, for this distribution); the kernel
computes max|.| on device and the host verifies it is exactly 7.0, falling
back to an exact numpy implementation otherwise.

Epilogue 2: y = clip(P2*(s2*sw2*inv2) + (x*inv2 + b2), +-1) with the residual
term done on the Scalar engine and a fused scalar_tensor_tensor on Vector.
"""

import numpy as np
import ml_dtypes

EPS = np.float32(1e-5)
NCORES = 8
B, C, H, W = 64, 256, 28, 28
BC = B // NCORES            # images per core
IMS = 912                   # padded (30x30=900) image stride, multiple of 16
NT = 420                    # matmul N: 14 padded rows x 30
MAGIC = np.float32(12582912.0)  # 1.5 * 2^23
F8NP = ml_dtypes.float8_e4m3

_BUILT = None  # cached (nc,) so repeat calls skip IR building


# ----------------------------------------------------------------- host math
def _quant_int(v):
    """Exact replica of the reference fake_quant grid; returns integer part."""
    alpha = np.float32(np.float32(np.max(np.abs(v))) + np.float32(1e-12))
    scale = np.float32(alpha / np.float32(7.0))
    q = np.round(np.clip(v, -alpha, alpha) / scale).astype(np.float32)
    return q, scale


def _fold_bn(gamma, beta, mean, var):
    inv = (gamma / np.sqrt(var + EPS)).astype(np.float32)
    b = (beta - mean * inv).astype(np.float32)
    return inv, b


# ------------------------------------------------------------------ bass IR
def _build():
    global _BUILT
    if _BUILT is not None:
        return _BUILT
    import concourse.bacc as bacc
    import concourse.tile as tile
    from concourse import mybir
    from contextlib import ExitStack

    f32 = mybir.dt.float32
    f8 = mybir.dt.float8e4
    AF = mybir.ActivationFunctionType
    OP = mybir.AluOpType
    DR = mybir.MatmulPerfMode.DoubleRow
    AX = mybir.AxisListType

    nc = bacc.Bacc("TRN2", target_bir_lowering=False, debug=False)
    x1_d = nc.dram_tensor("x1", [128, 2, BC, IMS], f8, kind="ExternalInput").ap()
    w_d = nc.dram_tensor("w", [128, 36, 2, 128], f8, kind="ExternalInput").ap()
    r_d = nc.dram_tensor("resid", [2, 128, BC, 2, 14, 28], f32, kind="ExternalInput").ap()
    v_d = nc.dram_tensor("vec", [128, 10], f32, kind="ExternalInput").ap()
    y_d = nc.dram_tensor("y", [2, 128, BC, 2, 14, 28], f32, kind="ExternalOutput").ap()
    am_d = nc.dram_tensor("amax", [128, 32], f32, kind="ExternalOutput").ap()

    with tile.TileContext(nc) as tc, ExitStack() as ctx:
        const = ctx.enter_context(tc.tile_pool(name="const", bufs=1))
        psum = ctx.enter_context(tc.tile_pool(name="psum", bufs=8, space="PSUM"))
        ep1 = ctx.enter_context(tc.tile_pool(name="ep1", bufs=3))
        ep2 = ctx.enter_context(tc.tile_pool(name="ep2", bufs=3))
        yp = ctx.enter_context(tc.tile_pool(name="yp", bufs=3))

        x1_sb = const.tile([128, 2, BC, IMS], f8, tag="x1")
        x2_sb = const.tile([128, 2, BC, IMS], f8, tag="x2")
        w_sb = const.tile([128, 36, 2, 128], f8, tag="w")
        rs_sb = const.tile([128, 2, BC, 2, 14, 28], f32, tag="rs")
        vec_sb = const.tile([128, 10], f32, tag="vec")
        am_sb = const.tile([128, 32], f32, tag="am")

        nc.sync.dma_start(x1_sb[:], x1_d)
        nc.sync.dma_start(w_sb[:], w_d)
        nc.sync.dma_start(vec_sb[:], v_d)
        nc.gpsimd.memset(x2_sb[:], 0.0)
        for cot in range(2):
            nc.sync.dma_start(rs_sb[:, cot], r_d[cot])

        def vcol(i):
            return vec_sb[:, i : i + 1]

        def valid(ap420):  # [128,420] -> [128,14,28] dropping 2 garbage cols/row
            return ap420.rearrange("p (h w) -> p h w", w=30)[:, :, :28]

        for ci, src_sb in ((0, x1_sb), (1, x2_sb)):
            for bg in range(4):
                pts = {}
                # ---- 18 accumulating DoubleRow matmuls per psum tile ----
                for cot in range(2):
                    for k in range(9):
                        off = (k // 3) * 30 + (k % 3)
                        lhsT = w_sb[:, ci * 18 + k * 2 + cot]
                        for bb in range(2):
                            b = bg * 2 + bb
                            for hb in range(2):
                                key = (cot, bb, hb)
                                if k == 0:
                                    pts[key] = psum.tile([128, NT], f32, tag="pt", name="pt")
                                s = hb * NT + off
                                nc.tensor.matmul(
                                    pts[key][:],
                                    lhsT,
                                    src_sb[:, :, b, s : s + NT],
                                    start=(k == 0),
                                    stop=(k == 8),
                                    perf_mode=DR,
                                )
                # ---- epilogues ----
                for cot in range(2):
                    for bb in range(2):
                        b = bg * 2 + bb
                        if ci == 1:
                            yb = yp.tile([128, 2, 14, 28], f32, tag="yb")
                        for hb in range(2):
                            pt3 = valid(pts[(cot, bb, hb)][:])
                            if ci == 0:
                                # t = P*a1 + b1p ; clip +-7 ; round-half-even -> fp8
                                t1 = ep1.tile([128, 14, 28], f32, tag="t1")
                                nc.scalar.activation(
                                    t1[:], pt3, AF.Identity,
                                    bias=vcol(2 + cot), scale=vcol(0 + cot))
                                t2 = ep1.tile([128, 14, 28], f32, tag="t2")
                                nc.vector.tensor_scalar(
                                    t2[:], t1[:], 7.0, -7.0, op0=OP.min, op1=OP.max)
                                idx = bg * 8 + cot * 4 + bb * 2 + hb
                                nc.vector.tensor_reduce(
                                    am_sb[:, idx : idx + 1], t2[:], op=OP.max,
                                    axis=AX.XY, apply_absolute_value=True)
                                t3 = ep1.tile([128, 14, 28], f32, tag="t3")
                                nc.scalar.activation(
                                    t3[:], t2[:], AF.Copy, bias=float(MAGIC), scale=1.0)
                                dst = valid(
                                    x2_sb[:, cot, b, hb * NT + 31 : hb * NT + 31 + NT])
                                nc.vector.tensor_scalar(
                                    dst, t3[:], -float(MAGIC), None, op0=OP.add)
                            else:
                                # y = clip(P2*c2 + (resid*inv2 + b2), +-1)
                                u2 = ep2.tile([128, 14, 28], f32, tag="u2")
                                nc.scalar.activation(
                                    u2[:], rs_sb[:, cot, b, hb], AF.Identity,
                                    bias=vcol(8 + cot), scale=vcol(6 + cot))
                                u3 = ep2.tile([128, 14, 28], f32, tag="u3")
                                nc.vector.scalar_tensor_tensor(
                                    u3[:], pt3, vcol(4 + cot), u2[:],
                                    op0=OP.mult, op1=OP.add)
                                nc.vector.tensor_scalar(
                                    yb[:, hb], u3[:], 1.0, -1.0, op0=OP.min, op1=OP.max)
                        if ci == 1:
                            nc.sync.dma_start(y_d[cot, :, b], yb[:])
        nc.sync.dma_start(am_d, am_sb[:])

    nc.compile()
    _BUILT = (nc,)
    return _BUILT


# ------------------------------------------------------------- input packing
def _prep(x, w1, w2, inv1, b1, inv2, b2):
    xi, s_x = _quant_int(x)
    w1i, s_w1 = _quant_int(w1)
    w2i, s_w2 = _quant_int(w2)

    xi8 = xi.astype(F8NP)
    tmp = np.zeros((NCORES, BC, 2, 128, 30, 30), F8NP)
    tmp[:, :, :, :, 1:29, 1:29] = xi8.reshape(NCORES, BC, 2, 128, 28, 28)
    x1_all = np.zeros((NCORES, 128, 2, BC, IMS), F8NP)
    x1_all[..., :900] = tmp.transpose(0, 3, 2, 1, 4, 5).reshape(
        NCORES, 128, 2, BC, 900)

    def wpack(wi):
        # w[cot*128+m, r*128+p, kh, kw] -> [p, (k,cot), r, m]
        v = wi.reshape(2, 128, 2, 128, 9)          # cot, m, r, p, k
        v = v.transpose(3, 4, 0, 2, 1)             # p, k, cot, r, m
        return v.reshape(128, 18, 2, 128).astype(F8NP)

    w_all = np.concatenate([wpack(w1i), wpack(w2i)], axis=1)  # [128,36,2,128]

    resid = x.reshape(NCORES, BC, 2, 128, 2, 14, 28).transpose(0, 2, 3, 1, 4, 5, 6)
    resid = np.ascontiguousarray(resid)            # [cores, 2, 128, BC, 2, 14, 28]

    s2 = np.float32(np.float32(1.0) / np.float32(7.0))
    a1 = (np.float32(7.0) * s_x * s_w1 * inv1).astype(np.float32)
    b1p = (np.float32(7.0) * b1).astype(np.float32)
    c2 = (s2 * s_w2 * inv2).astype(np.float32)
    cols = [a1[:128], a1[128:], b1p[:128], b1p[128:], c2[:128], c2[128:],
            inv2[:128], inv2[128:], b2[:128], b2[128:]]
    vec = np.ascontiguousarray(np.stack(cols, axis=1).astype(np.float32))

    in_maps = [
        {"x1": x1_all[i], "w": w_all, "resid": resid[i], "vec": vec}
        for i in range(NCORES)
    ]
    return in_maps, (xi, w1i, w2i, s_x, s_w1, s_w2, s2)


# ------------------------------------------------------- exact numpy fallback
def _conv3x3_int(xint, wint):
    Bn, Cn, Hn, Wn = xint.shape
    xp = np.zeros((Bn, Cn, Hn + 2, Wn + 2), np.float64)
    xp[:, :, 1:-1, 1:-1] = xint
    out = np.zeros((Bn, wint.shape[0], Hn, Wn), np.float64)
    w64 = wint.astype(np.float64)
    for kh in range(3):
        for kw in range(3):
            out += np.einsum("bchw,oc->bohw", xp[:, :, kh:kh + Hn, kw:kw + Wn],
                             w64[:, :, kh, kw], optimize=True)
    return out.astype(np.float32)


def _numpy_path(x, q, inv1, b1, inv2, b2):
    """Exact replica handling arbitrary alpha2 (never expected to run)."""
    xi, w1i, w2i, s_x, s_w1, s_w2, _ = q
    P1 = _conv3x3_int(xi, w1i)
    h = (P1 * (s_x * s_w1 * inv1)[None, :, None, None]).astype(np.float32)
    h = (h + b1[None, :, None, None]).astype(np.float32)
    h = np.clip(h, np.float32(-1.0), np.float32(1.0))
    alpha2 = np.float32(np.abs(h).max())
    s2 = np.float32(alpha2 / np.float32(7.0))
    x2 = np.round(np.clip(h, -alpha2, alpha2) / s2).astype(np.float32)
    P2 = _conv3x3_int(x2, w2i)
    u = (P2 * (s2 * s_w2 * inv2)[None, :, None, None]).astype(np.float32)
    u = (u + (x * inv2[None, :, None, None] + b2[None, :, None, None])).astype(np.float32)
    return np.clip(u, np.float32(-1.0), np.float32(1.0))


# ------------------------------------------------------------------- kernel
def _run(in_maps, trace=False, tmpdir=None):
    from concourse.bass_utils import run_bass_kernel_spmd
    (nc,) = _build()
    return run_bass_kernel_spmd(nc, in_maps, list(range(NCORES)), trace=trace,
                                tmpdir=tmpdir)


def kernel(x, w1, bn1_gamma, bn1_beta, bn1_mean, bn1_var,
           w2, bn2_gamma, bn2_beta, bn2_mean, bn2_var):
    x = np.asarray(x, np.float32)
    w1 = np.asarray(w1, np.float32)
    w2 = np.asarray(w2, np.float32)
    inv1, b1 = _fold_bn(np.asarray(bn1_gamma, np.float32), np.asarray(bn1_beta, np.float32),
                        np.asarray(bn1_mean, np.float32), np.asarray(bn1_var, np.float32))
    inv2, b2 = _fold_bn(np.asarray(bn2_gamma, np.float32), np.asarray(bn2_beta, np.float32),
                        np.asarray(bn2_mean, np.float32), np.asarray(bn2_var, np.float32))

    in_maps, q = _prep(x, w1, w2, inv1, b1, inv2, b2)
    res = _run(in_maps)

    amax = np.max([r["amax"] for r in res.results])
    if not np.float32(amax) == np.float32(7.0):
        return _numpy_path(x, q, inv1, b1, inv2, b2)

    ys = np.stack([r["y"] for r in res.results])      # [cores, 2, 128, BC, 2,14,28]
    ys = ys.reshape(NCORES, 2, 128, BC, 784)
    return ys.transpose(0, 3, 1, 2, 4).reshape(B, C, H, W).copy()
